# revision 2
# baseline (speedup 1.0000x reference)
"""Split-side Trainium2 kernel for BondPoolingLayer.

Architecture (single SPMD program, per-core variation lives in data only):
  - Node space padded to 200704 rows = 56 blocks x 3584. Stream block b is
    one contiguous-per-partition DMA (partition p holds rows 3584b+28p..+28,
    7KB descriptors -> full DMA bandwidth). Chunk (b,j), j in [0,28), is the
    128-row set {3584b + 28p + j}; its chunk-local id of node n is the
    partition p. Band = 7 consecutive chunks (896 nodes), 4 bands/block,
    224 bands total.
  - Edges are binned by src band; each band owns 128 fixed edge slots
    (mean occupancy 123/128). Overflow edges go to 2 spill tiles (1024
    slots). Slot layout is FIXED (program structure identical on all
    cores); which edge sits in which slot is data.
  - src side (banded slots): one-hot selection matmuls. S is built on-chip:
    DVE is_equal of the broadcast slot-request tile (req = 128*(j%7) + p,
    f16) against f32 scalars (128k + p). For tile t (= block t, 4 bands,
    512 slots): 28 matmuls, stationary H_chunk [128 nodes, 128 feat] fp16,
    rhs S slice [128, 128] -> feature-major psum. Zero Pool time.
  - dst side + spill src: SWDGE indirect DMA gather of h fp16 rows (240
    calls x 128 rows ~ 1.4us Pool each, vs 432 in the baseline), then PE
    transpose to feature-major.
  - MLP fp16 weights/acts, fp32 psum, per 512-slot tile. Host maps slots
    back to edges.
"""

import numpy as np

import concourse.bass as bass
import concourse.mybir as mybir
import concourse.tile as tile
from concourse import bacc
from concourse.bass import IndirectOffsetOnAxis
from concourse.bass_utils import run_bass_kernel_spmd
from concourse.masks import make_identity

N_NODES = 200000
D = 128
E_TOTAL = 220000
N_CORES = 8
E_CORE = E_TOTAL // N_CORES      # 27500

ROWS_PP = 28                      # rows per partition per block
BLK_ROWS = 128 * ROWS_PP          # 3584
N_BLOCKS = 56
N_PAD = N_BLOCKS * BLK_ROWS       # 200704
BAND_CH = 7                       # chunks per band
BANDS_PB = ROWS_PP // BAND_CH     # 4 bands per block
N_BANDS = N_BLOCKS * BANDS_PB     # 224
BAND_NODES = BAND_CH * 128        # 896

CAP = 128                         # edge slots per band
TILE_E = BANDS_PB * CAP           # 512 slots per tile
N_BTILES = N_BLOCKS               # 56 banded tiles
N_SPILL_T = 2                     # spill tiles
SPILL_SLOTS = N_SPILL_T * TILE_E  # 1024
N_TILES = N_BTILES + N_SPILL_T    # 58
E_PAD = N_TILES * TILE_E          # 29696
DUMMY_REQ = 4096.0                # matches nothing (< 128*7 never)

F32 = mybir.dt.float32
F16 = mybir.dt.float16
I32 = mybir.dt.int32


def build_nc():
    nc = bacc.Bacc("TRN2", target_bir_lowering=False, debug=False)

    h = nc.dram_tensor("h", [N_PAD, D], F16, kind="ExternalInput")
    didx = nc.dram_tensor("didx", [128, E_PAD // 128], I32, kind="ExternalInput")
    sidx = nc.dram_tensor("sidx", [128, SPILL_SLOTS // 128], I32,
                          kind="ExternalInput")
    sreq = nc.dram_tensor("sreq", [1, N_BTILES * TILE_E], F16,
                          kind="ExternalInput")
    kiota = nc.dram_tensor("kiota", [128, BAND_CH], F32, kind="ExternalInput")
    w1a = nc.dram_tensor("w1a", [128, 128], F16, kind="ExternalInput")
    w1b = nc.dram_tensor("w1b", [128, 128], F16, kind="ExternalInput")
    w2 = nc.dram_tensor("w2", [128, 128], F16, kind="ExternalInput")
    w3 = nc.dram_tensor("w3", [128, 2], F16, kind="ExternalInput")
    b1 = nc.dram_tensor("b1", [128, 1], F32, kind="ExternalInput")
    b2 = nc.dram_tensor("b2", [128, 1], F32, kind="ExternalInput")
    b3x2 = nc.dram_tensor("b3x2", [2, 1], F32, kind="ExternalInput")
    out = nc.dram_tensor("out", [2, E_PAD], F32, kind="ExternalOutput")

    with tile.TileContext(nc) as tc:
        _program(tc, nc, h, didx, sidx, sreq, kiota,
                 w1a, w1b, w2, w3, b1, b2, b3x2, out)
    nc.compile()
    return nc


def _program(tc, nc, h, didx, sidx, sreq, kiota,
             w1a, w1b, w2, w3, b1, b2, b3x2, out):
    Relu = mybir.ActivationFunctionType.Relu
    EQ = mybir.AluOpType.is_equal

    with (
        tc.tile_pool(name="const", bufs=1) as const_pool,
        tc.tile_pool(name="stream", bufs=3) as stream_pool,
        tc.tile_pool(name="req", bufs=2) as req_pool,
        tc.tile_pool(name="sel", bufs=2) as sel_pool,
        tc.tile_pool(name="gat", bufs=2) as gat_pool,
        tc.tile_pool(name="xbuf", bufs=3) as x_pool,
        tc.tile_pool(name="act", bufs=3) as act_pool,
        tc.tile_pool(name="outp", bufs=4) as out_pool,
        tc.tile_pool(name="xsp", bufs=2, space="PSUM") as xs_psum,
        tc.tile_pool(name="trp", bufs=2, space="PSUM") as tr_psum,
        tc.tile_pool(name="l1p", bufs=1, space="PSUM") as l1_psum,
        tc.tile_pool(name="l2p", bufs=1, space="PSUM") as l2_psum,
    ):
        # ---- constants ----
        ident = const_pool.tile([128, 128], F16)
        make_identity(nc, ident[:])
        w1a_t = const_pool.tile([128, 128], F16)
        nc.sync.dma_start(w1a_t[:], w1a.ap())
        w1b_t = const_pool.tile([128, 128], F16)
        nc.sync.dma_start(w1b_t[:], w1b.ap())
        w2_t = const_pool.tile([128, 128], F16)
        nc.sync.dma_start(w2_t[:], w2.ap())
        w3_t = const_pool.tile([128, 2], F16)
        nc.sync.dma_start(w3_t[:], w3.ap())
        b1_t = const_pool.tile([128, 1], F32)
        nc.sync.dma_start(b1_t[:], b1.ap())
        b2_t = const_pool.tile([128, 1], F32)
        nc.sync.dma_start(b2_t[:], b2.ap())
        b3_t = const_pool.tile([2, 1], F32)
        nc.sync.dma_start(b3_t[:], b3x2.ap())
        kio_t = const_pool.tile([128, BAND_CH], F32)
        nc.sync.dma_start(kio_t[:], kiota.ap())
        didx_t = const_pool.tile([128, E_PAD // 128], I32)
        nc.sync.dma_start(didx_t[:], didx.ap())
        sidx_t = const_pool.tile([128, SPILL_SLOTS // 128], I32)
        nc.sync.dma_start(sidx_t[:], sidx.ap())

        # DRAM h viewed per stream block: partition p <- rows 3584b+28p..+28
        h_blk = h.ap().rearrange("(b p k) d -> b p k d", p=128, k=ROWS_PP)
        sreq_ap = sreq.ap()
        out_ap = out.ap()

        def mlp_tile(t, xsT, xdT):
            l1 = l1_psum.tile([128, 2 * TILE_E], F32, tag="l1", space="PSUM")
            nc.tensor.matmul(l1[:, 0:TILE_E], w1a_t[:], xsT, start=True, stop=False)
            nc.tensor.matmul(l1[:, 0:TILE_E], w1b_t[:], xdT, start=False, stop=True)
            nc.tensor.matmul(l1[:, TILE_E:], w1a_t[:], xdT, start=True, stop=False)
            nc.tensor.matmul(l1[:, TILE_E:], w1b_t[:], xsT, start=False, stop=True)

            h1 = act_pool.tile([128, 2 * TILE_E], F16, tag="h1")
            nc.scalar.activation(h1[:], l1[:], Relu, bias=b1_t[:, 0:1])

            l2 = l2_psum.tile([128, 2 * TILE_E], F32, tag="l2", space="PSUM")
            nc.tensor.matmul(l2[:, 0:TILE_E], w2_t[:], h1[:, 0:TILE_E],
                             start=True, stop=True)
            nc.tensor.matmul(l2[:, TILE_E:], w2_t[:], h1[:, TILE_E:],
                             start=True, stop=True)

            h2 = act_pool.tile([128, 2 * TILE_E], F16, tag="h2")
            nc.scalar.activation(h2[:], l2[:], Relu, bias=b2_t[:, 0:1])

            l3 = l2_psum.tile([2, TILE_E], F32, tag="l2", space="PSUM")
            nc.tensor.matmul(l3[:], w3_t[:], h2[:, 0:TILE_E], start=True, stop=False)
            nc.tensor.matmul(l3[:], w3_t[:], h2[:, TILE_E:], start=False, stop=True)

            o = out_pool.tile([2, TILE_E], F32, tag="o")
            nc.vector.tensor_scalar_add(o[:], l3[:], b3_t[:, 0:1])
            nc.sync.dma_start(out_ap[:, t * TILE_E:(t + 1) * TILE_E], o[:])

        def gather_transposed(idx_tile, col0, dst_sb):
            """Gather 4x128 h rows (fp16) by idx columns col0..col0+4, PE
            transpose to feature-major, DVE copy into dst_sb [128, TILE_E]."""
            gt = gat_pool.tile([128, 4, D], F16, tag="g")
            for gj in range(4):
                nc.gpsimd.indirect_dma_start(
                    out=gt[:, gj, :],
                    out_offset=None,
                    in_=h.ap(),
                    in_offset=IndirectOffsetOnAxis(
                        ap=idx_tile[:, col0 + gj:col0 + gj + 1], axis=0),
                )
            trp = tr_psum.tile([128, TILE_E], F16, tag="tr", space="PSUM")
            for gj in range(4):
                nc.tensor.transpose(
                    trp[:, gj * 128:(gj + 1) * 128], gt[:, gj, :], ident[:])
            nc.vector.tensor_copy(dst_sb[:], trp[:])

        # ---- banded tiles ----
        for t in range(N_BTILES):
            st = stream_pool.tile([128, ROWS_PP, D], F16, tag="sb")
            nc.sync.dma_start(st[:], h_blk[t])

            rq = req_pool.tile([128, TILE_E], F16, tag="rq")
            nc.sync.dma_start(
                rq[:],
                sreq_ap[:, t * TILE_E:(t + 1) * TILE_E].broadcast_to(
                    [128, TILE_E]))

            s_t = sel_pool.tile([128, BAND_CH, TILE_E], F16, tag="s")
            for k in range(BAND_CH):
                nc.vector.tensor_scalar(s_t[:, k, :], rq[:], kio_t[:, k:k + 1],
                                        None, EQ)

            xd = x_pool.tile([128, TILE_E], F16, tag="xd")
            gather_transposed(didx_t, t * 4, xd)

            xsp = xs_psum.tile([128, TILE_E], F32, tag="xsp", space="PSUM")
            for jg in range(BANDS_PB):
                for k in range(BAND_CH):
                    j = jg * BAND_CH + k
                    nc.tensor.matmul(
                        xsp[:, jg * CAP:(jg + 1) * CAP],
                        st[:, j, :],
                        s_t[:, k, jg * CAP:(jg + 1) * CAP],
                        start=(k == 0), stop=(k == BAND_CH - 1))
            xs = x_pool.tile([128, TILE_E], F16, tag="xs")
            nc.vector.tensor_copy(xs[:], xsp[:])

            mlp_tile(t, xs[:], xd[:])

        # ---- spill tiles: both sides gathered ----
        for i in range(N_SPILL_T):
            t = N_BTILES + i
            xs = x_pool.tile([128, TILE_E], F16, tag="xs")
            gather_transposed(sidx_t, i * 4, xs)
            xd = x_pool.tile([128, TILE_E], F16, tag="xd")
            gather_transposed(didx_t, t * 4, xd)
            mlp_tile(t, xs[:], xd[:])


_NC_CACHE = {}


def _get_nc():
    if "nc" not in _NC_CACHE:
        _NC_CACHE["nc"] = build_nc()
    return _NC_CACHE["nc"]


def _node_meta(n):
    """(band, req_local) of node id n under the block/chunk layout."""
    b, r = divmod(n, BLK_ROWS)
    p, j = divmod(r, ROWS_PP)
    band = b * BANDS_PB + j // BAND_CH
    req = 128 * (j % BAND_CH) + p
    return band, req


def assign_slots(src):
    """Bin edges by src band into fixed 128-cap slots; overflow -> spill.
    Returns (slot_of_edge [n], spill_edges list)."""
    n = len(src)
    band, req = _node_meta(src)
    slot = np.full(n, -1, dtype=np.int64)
    counts = np.zeros(N_BANDS, dtype=np.int64)
    spill = []
    for e in range(n):
        b = band[e]
        if counts[b] < CAP:
            slot[e] = b * CAP + counts[b]
            counts[b] += 1
        else:
            spill.append(e)
    if len(spill) > SPILL_SLOTS:
        raise RuntimeError(f"spill overflow: {len(spill)} > {SPILL_SLOTS}")
    base = N_BTILES * TILE_E
    for i, e in enumerate(spill):
        slot[e] = base + i
    return slot, req


def _wrap_idx(vals, n_slots):
    """[n_slots] int -> [128, n_slots//128] int32 with tile j partition p
    holding vals[128*j + p]."""
    return np.ascontiguousarray(
        vals.reshape(n_slots // 128, 128).T.astype(np.int32))


def make_in_map(h16, src_shard, dst_shard, W1, b1, W2, b2, W3, b3):
    src = np.asarray(src_shard, dtype=np.int64)
    dst = np.asarray(dst_shard, dtype=np.int64)
    slot, req_local = assign_slots(src)

    nb = N_BTILES * TILE_E
    sreq = np.full(nb, DUMMY_REQ, dtype=np.float16)
    didx_full = np.zeros(E_PAD, dtype=np.int64)
    sidx_spill = np.zeros(SPILL_SLOTS, dtype=np.int64)

    banded = slot < nb
    sreq[slot[banded]] = req_local[banded].astype(np.float16)
    didx_full[slot] = dst
    sp = ~banded
    sidx_spill[slot[sp] - nb] = src[sp]

    kio = (np.arange(128, dtype=np.float32)[:, None]
           + 128.0 * np.arange(BAND_CH, dtype=np.float32)[None, :])

    return {
        "h": h16,
        "didx": _wrap_idx(didx_full, E_PAD),
        "sidx": _wrap_idx(sidx_spill, SPILL_SLOTS),
        "sreq": np.ascontiguousarray(sreq.reshape(1, nb)),
        "kiota": np.ascontiguousarray(kio),
        "w1a": np.ascontiguousarray(W1[:128], dtype=np.float16),
        "w1b": np.ascontiguousarray(W1[128:], dtype=np.float16),
        "w2": np.ascontiguousarray(W2, dtype=np.float16),
        "w3": np.ascontiguousarray(W3, dtype=np.float16),
        "b1": np.ascontiguousarray(np.asarray(b1).reshape(128, 1), dtype=np.float32),
        "b2": np.ascontiguousarray(np.asarray(b2).reshape(128, 1), dtype=np.float32),
        "b3x2": np.ascontiguousarray(
            (2.0 * np.asarray(b3)).reshape(2, 1), dtype=np.float32),
    }, slot


def kernel(h, src, dst, W1, b1, W2, b2, W3, b3, **run_kwargs):
    h = np.asarray(h, dtype=np.float32)
    src = np.asarray(src).astype(np.int64)
    dst = np.asarray(dst).astype(np.int64)
    W1 = np.asarray(W1); W2 = np.asarray(W2); W3 = np.asarray(W3)
    b1 = np.asarray(b1); b2 = np.asarray(b2); b3 = np.asarray(b3)

    h16 = np.zeros((N_PAD, D), dtype=np.float16)
    h16[:N_NODES] = h.astype(np.float16)

    nc = _get_nc()
    in_maps, slots = [], []
    for c in range(N_CORES):
        sl = slice(c * E_CORE, (c + 1) * E_CORE)
        im, slot = make_in_map(h16, src[sl], dst[sl], W1, b1, W2, b2, W3, b3)
        in_maps.append(im)
        slots.append(slot)

    try:
        res = run_bass_kernel_spmd(nc, in_maps, core_ids=list(range(N_CORES)),
                                   **run_kwargs)
    except Exception:
        import time as _time
        _time.sleep(5)
        res = run_bass_kernel_spmd(nc, in_maps, core_ids=list(range(N_CORES)),
                                   **run_kwargs)

    out = np.empty((E_TOTAL, 2), dtype=np.float32)
    for c in range(N_CORES):
        o = res.results[c]["out"]          # [2, E_PAD]
        out[c * E_CORE:(c + 1) * E_CORE] = o.T[slots[c]]
    if run_kwargs:
        kernel.last_results = res
    return out


# revision 3
# speedup vs baseline: 1.0167x; 1.0167x over previous
"""Split-side Trainium2 kernel for BondPoolingLayer.

Architecture (single SPMD program, per-core variation lives in data only):
  - Node space padded to 200704 rows = 56 blocks x 3584. Stream block b is
    one contiguous-per-partition DMA (partition p holds rows 3584b+28p..+28,
    7KB descriptors -> full DMA bandwidth). Chunk (b,j), j in [0,28), is the
    128-row set {3584b + 28p + j}; its chunk-local id of node n is the
    partition p. Band = 7 consecutive chunks (896 nodes), 4 bands/block,
    224 bands total.
  - Edges are binned by src band; each band owns 128 fixed edge slots
    (mean occupancy 123/128). Overflow edges go to 2 spill tiles (1024
    slots). Slot layout is FIXED (program structure identical on all
    cores); which edge sits in which slot is data.
  - src side (banded slots): one-hot selection matmuls. S is built on-chip:
    DVE is_equal of the broadcast slot-request tile (req = 128*(j%7) + p,
    f16) against f32 scalars (128k + p). For tile t (= block t, 4 bands,
    512 slots): 28 matmuls, stationary H_chunk [128 nodes, 128 feat] fp16,
    rhs S slice [128, 128] -> feature-major psum. Zero Pool time.
  - dst side + spill src: SWDGE indirect DMA gather of h fp16 rows (240
    calls x 128 rows ~ 1.4us Pool each, vs 432 in the baseline), then PE
    transpose to feature-major.
  - MLP fp16 weights/acts, fp32 psum, per 512-slot tile. Host maps slots
    back to edges.
"""

import numpy as np

import concourse.bass as bass
import concourse.mybir as mybir
import concourse.tile as tile
from concourse import bacc
from concourse.bass import IndirectOffsetOnAxis
from concourse.bass_utils import run_bass_kernel_spmd
from concourse.masks import make_identity

N_NODES = 200000
D = 128
E_TOTAL = 220000
N_CORES = 8
E_CORE = E_TOTAL // N_CORES      # 27500

ROWS_PP = 28                      # rows per partition per block
BLK_ROWS = 128 * ROWS_PP          # 3584
N_BLOCKS = 56
N_PAD = N_BLOCKS * BLK_ROWS       # 200704
BAND_CH = 7                       # chunks per band
BANDS_PB = ROWS_PP // BAND_CH     # 4 bands per block
N_BANDS = N_BLOCKS * BANDS_PB     # 224
BAND_NODES = BAND_CH * 128        # 896

CAP = 128                         # edge slots per band
TILE_E = BANDS_PB * CAP           # 512 slots per tile
N_BTILES = N_BLOCKS               # 56 banded tiles
N_SPILL_T = 2                     # spill tiles
SPILL_SLOTS = N_SPILL_T * TILE_E  # 1024
N_TILES = N_BTILES + N_SPILL_T    # 58
E_PAD = N_TILES * TILE_E          # 29696
DUMMY_REQ = 4096.0                # matches nothing (< 128*7 never)

F32 = mybir.dt.float32
F16 = mybir.dt.float16
I32 = mybir.dt.int32


def build_nc():
    nc = bacc.Bacc("TRN2", target_bir_lowering=False, debug=False)

    h = nc.dram_tensor("h", [N_PAD, D], F16, kind="ExternalInput")
    didx = nc.dram_tensor("didx", [128, E_PAD // 128], I32, kind="ExternalInput")
    sidx = nc.dram_tensor("sidx", [128, SPILL_SLOTS // 128], I32,
                          kind="ExternalInput")
    sreq = nc.dram_tensor("sreq", [1, N_BTILES * TILE_E], F16,
                          kind="ExternalInput")
    kiota = nc.dram_tensor("kiota", [128, BAND_CH], F32, kind="ExternalInput")
    w1a = nc.dram_tensor("w1a", [128, 128], F16, kind="ExternalInput")
    w1b = nc.dram_tensor("w1b", [128, 128], F16, kind="ExternalInput")
    w2 = nc.dram_tensor("w2", [128, 128], F16, kind="ExternalInput")
    w3 = nc.dram_tensor("w3", [128, 2], F16, kind="ExternalInput")
    b1 = nc.dram_tensor("b1", [128, 1], F32, kind="ExternalInput")
    b2 = nc.dram_tensor("b2", [128, 1], F32, kind="ExternalInput")
    b3x2 = nc.dram_tensor("b3x2", [2, 1], F32, kind="ExternalInput")
    out = nc.dram_tensor("out", [2, E_PAD], F32, kind="ExternalOutput")

    with tile.TileContext(nc) as tc:
        _program(tc, nc, h, didx, sidx, sreq, kiota,
                 w1a, w1b, w2, w3, b1, b2, b3x2, out)
    nc.compile()
    return nc


def _program(tc, nc, h, didx, sidx, sreq, kiota,
             w1a, w1b, w2, w3, b1, b2, b3x2, out):
    Relu = mybir.ActivationFunctionType.Relu
    EQ = mybir.AluOpType.is_equal

    with (
        tc.tile_pool(name="const", bufs=1) as const_pool,
        tc.tile_pool(name="stream", bufs=3) as stream_pool,
        tc.tile_pool(name="req", bufs=2) as req_pool,
        tc.tile_pool(name="sel", bufs=2) as sel_pool,
        tc.tile_pool(name="gat", bufs=4) as gat_pool,
        tc.tile_pool(name="xbuf", bufs=6) as x_pool,
        tc.tile_pool(name="act", bufs=3) as act_pool,
        tc.tile_pool(name="outp", bufs=4) as out_pool,
        tc.tile_pool(name="xsp", bufs=2, space="PSUM") as xs_psum,
        tc.tile_pool(name="trp", bufs=2, space="PSUM") as tr_psum,
        tc.tile_pool(name="l1p", bufs=1, space="PSUM") as l1_psum,
        tc.tile_pool(name="l2p", bufs=1, space="PSUM") as l2_psum,
    ):
        # ---- constants ----
        ident = const_pool.tile([128, 128], F16)
        make_identity(nc, ident[:])
        w1a_t = const_pool.tile([128, 128], F16)
        nc.sync.dma_start(w1a_t[:], w1a.ap())
        w1b_t = const_pool.tile([128, 128], F16)
        nc.sync.dma_start(w1b_t[:], w1b.ap())
        w2_t = const_pool.tile([128, 128], F16)
        nc.sync.dma_start(w2_t[:], w2.ap())
        w3_t = const_pool.tile([128, 2], F16)
        nc.sync.dma_start(w3_t[:], w3.ap())
        b1_t = const_pool.tile([128, 1], F32)
        nc.sync.dma_start(b1_t[:], b1.ap())
        b2_t = const_pool.tile([128, 1], F32)
        nc.sync.dma_start(b2_t[:], b2.ap())
        b3_t = const_pool.tile([2, 1], F32)
        nc.sync.dma_start(b3_t[:], b3x2.ap())
        kio_t = const_pool.tile([128, BAND_CH], F32)
        nc.sync.dma_start(kio_t[:], kiota.ap())
        didx_t = const_pool.tile([128, E_PAD // 128], I32)
        nc.sync.dma_start(didx_t[:], didx.ap())
        sidx_t = const_pool.tile([128, SPILL_SLOTS // 128], I32)
        nc.sync.dma_start(sidx_t[:], sidx.ap())

        # DRAM h viewed per stream block: partition p <- rows 3584b+28p..+28
        h_blk = h.ap().rearrange("(b p k) d -> b p k d", p=128, k=ROWS_PP)
        sreq_ap = sreq.ap()
        out_ap = out.ap()

        def mlp_tile(t, xsT, xdT):
            l1f = l1_psum.tile([128, TILE_E], F32, tag="l1f", space="PSUM")
            nc.tensor.matmul(l1f[:], w1a_t[:], xsT, start=True, stop=False)
            nc.tensor.matmul(l1f[:], w1b_t[:], xdT, start=False, stop=True)
            h1f = act_pool.tile([128, TILE_E], F16, tag="h1f")
            nc.scalar.activation(h1f[:], l1f[:], Relu, bias=b1_t[:, 0:1])

            l1r = l1_psum.tile([128, TILE_E], F32, tag="l1r", space="PSUM")
            nc.tensor.matmul(l1r[:], w1a_t[:], xdT, start=True, stop=False)
            nc.tensor.matmul(l1r[:], w1b_t[:], xsT, start=False, stop=True)
            h1r = act_pool.tile([128, TILE_E], F16, tag="h1r")
            nc.scalar.activation(h1r[:], l1r[:], Relu, bias=b1_t[:, 0:1])

            l2f = l2_psum.tile([128, TILE_E], F32, tag="l2f", space="PSUM")
            nc.tensor.matmul(l2f[:], w2_t[:], h1f[:], start=True, stop=True)
            h2f = act_pool.tile([128, TILE_E], F16, tag="h2f")
            nc.scalar.activation(h2f[:], l2f[:], Relu, bias=b2_t[:, 0:1])

            l2r = l2_psum.tile([128, TILE_E], F32, tag="l2r", space="PSUM")
            nc.tensor.matmul(l2r[:], w2_t[:], h1r[:], start=True, stop=True)
            h2r = act_pool.tile([128, TILE_E], F16, tag="h2r")
            nc.scalar.activation(h2r[:], l2r[:], Relu, bias=b2_t[:, 0:1])

            l3 = l2_psum.tile([2, TILE_E], F32, tag="l3", space="PSUM")
            nc.tensor.matmul(l3[:], w3_t[:], h2f[:], start=True, stop=False)
            nc.tensor.matmul(l3[:], w3_t[:], h2r[:], start=False, stop=True)

            o = out_pool.tile([2, TILE_E], F32, tag="o")
            nc.vector.tensor_scalar_add(o[:], l3[:], b3_t[:, 0:1])
            nc.sync.dma_start(out_ap[:, t * TILE_E:(t + 1) * TILE_E], o[:])

        def gather_issue(idx_tile, col0):
            gt = gat_pool.tile([128, 4, D], F16, tag="g")
            for gj in range(4):
                nc.gpsimd.indirect_dma_start(
                    out=gt[:, gj, :],
                    out_offset=None,
                    in_=h.ap(),
                    in_offset=IndirectOffsetOnAxis(
                        ap=idx_tile[:, col0 + gj:col0 + gj + 1], axis=0),
                )
            return gt

        def gather_consume(gt, dst_sb):
            trp = tr_psum.tile([128, TILE_E], F16, tag="tr", space="PSUM")
            for gj in range(4):
                nc.tensor.transpose(
                    trp[:, gj * 128:(gj + 1) * 128], gt[:, gj, :], ident[:])
            nc.vector.tensor_copy(dst_sb[:], trp[:])

        # gather issue schedule: (idx_tile, col0) per tile, banded then spill
        gplan = [(didx_t, t * 4) for t in range(N_BTILES)]
        for i in range(N_SPILL_T):
            gplan.append((sidx_t, i * 4))                 # spill src
            gplan.append((didx_t, (N_BTILES + i) * 4))    # spill dst
        PREFETCH = 2
        pending = {}
        for gi in range(PREFETCH):
            pending[gi] = gather_issue(*gplan[gi])
        next_gi = PREFETCH

        def fetch(gi, dst_sb):
            nonlocal next_gi
            gt = pending.pop(gi)
            if next_gi < len(gplan):
                pending[next_gi] = gather_issue(*gplan[next_gi])
                next_gi += 1
            gather_consume(gt, dst_sb)

        # ---- banded tiles ----
        for t in range(N_BTILES):
            st = stream_pool.tile([128, ROWS_PP, D], F16, tag="sb")
            nc.sync.dma_start(st[:], h_blk[t])

            rq = req_pool.tile([128, TILE_E], F16, tag="rq")
            nc.sync.dma_start(
                rq[:],
                sreq_ap[:, t * TILE_E:(t + 1) * TILE_E].broadcast_to(
                    [128, TILE_E]))

            s_t = sel_pool.tile([128, BAND_CH, TILE_E], F16, tag="s")
            for k in range(BAND_CH):
                nc.vector.tensor_scalar(s_t[:, k, :], rq[:], kio_t[:, k:k + 1],
                                        None, EQ)

            xd = x_pool.tile([128, TILE_E], F16, tag="xd")
            fetch(t, xd)

            xsp = xs_psum.tile([128, TILE_E], F32, tag="xsp", space="PSUM")
            for jg in range(BANDS_PB):
                for k in range(BAND_CH):
                    j = jg * BAND_CH + k
                    nc.tensor.matmul(
                        xsp[:, jg * CAP:(jg + 1) * CAP],
                        st[:, j, :],
                        s_t[:, k, jg * CAP:(jg + 1) * CAP],
                        start=(k == 0), stop=(k == BAND_CH - 1))
            xs = x_pool.tile([128, TILE_E], F16, tag="xs")
            nc.vector.tensor_copy(xs[:], xsp[:])

            mlp_tile(t, xs[:], xd[:])

        # ---- spill tiles: both sides gathered ----
        for i in range(N_SPILL_T):
            t = N_BTILES + i
            xs = x_pool.tile([128, TILE_E], F16, tag="xs")
            fetch(N_BTILES + 2 * i, xs)
            xd = x_pool.tile([128, TILE_E], F16, tag="xd")
            fetch(N_BTILES + 2 * i + 1, xd)
            mlp_tile(t, xs[:], xd[:])


_NC_CACHE = {}


def _get_nc():
    if "nc" not in _NC_CACHE:
        _NC_CACHE["nc"] = build_nc()
    return _NC_CACHE["nc"]


def _node_meta(n):
    """(band, req_local) of node id n under the block/chunk layout."""
    b, r = divmod(n, BLK_ROWS)
    p, j = divmod(r, ROWS_PP)
    band = b * BANDS_PB + j // BAND_CH
    req = 128 * (j % BAND_CH) + p
    return band, req


def assign_slots(src):
    """Bin edges by src band into fixed 128-cap slots; overflow -> spill.
    Returns (slot_of_edge [n], spill_edges list)."""
    n = len(src)
    band, req = _node_meta(src)
    slot = np.full(n, -1, dtype=np.int64)
    counts = np.zeros(N_BANDS, dtype=np.int64)
    spill = []
    for e in range(n):
        b = band[e]
        if counts[b] < CAP:
            slot[e] = b * CAP + counts[b]
            counts[b] += 1
        else:
            spill.append(e)
    if len(spill) > SPILL_SLOTS:
        raise RuntimeError(f"spill overflow: {len(spill)} > {SPILL_SLOTS}")
    base = N_BTILES * TILE_E
    for i, e in enumerate(spill):
        slot[e] = base + i
    return slot, req


def _wrap_idx(vals, n_slots):
    """[n_slots] int -> [128, n_slots//128] int32 with tile j partition p
    holding vals[128*j + p]."""
    return np.ascontiguousarray(
        vals.reshape(n_slots // 128, 128).T.astype(np.int32))


def make_in_map(h16, src_shard, dst_shard, W1, b1, W2, b2, W3, b3):
    src = np.asarray(src_shard, dtype=np.int64)
    dst = np.asarray(dst_shard, dtype=np.int64)
    slot, req_local = assign_slots(src)

    nb = N_BTILES * TILE_E
    sreq = np.full(nb, DUMMY_REQ, dtype=np.float16)
    didx_full = np.zeros(E_PAD, dtype=np.int64)
    sidx_spill = np.zeros(SPILL_SLOTS, dtype=np.int64)

    banded = slot < nb
    sreq[slot[banded]] = req_local[banded].astype(np.float16)
    didx_full[slot] = dst
    sp = ~banded
    sidx_spill[slot[sp] - nb] = src[sp]

    kio = (np.arange(128, dtype=np.float32)[:, None]
           + 128.0 * np.arange(BAND_CH, dtype=np.float32)[None, :])

    return {
        "h": h16,
        "didx": _wrap_idx(didx_full, E_PAD),
        "sidx": _wrap_idx(sidx_spill, SPILL_SLOTS),
        "sreq": np.ascontiguousarray(sreq.reshape(1, nb)),
        "kiota": np.ascontiguousarray(kio),
        "w1a": np.ascontiguousarray(W1[:128], dtype=np.float16),
        "w1b": np.ascontiguousarray(W1[128:], dtype=np.float16),
        "w2": np.ascontiguousarray(W2, dtype=np.float16),
        "w3": np.ascontiguousarray(W3, dtype=np.float16),
        "b1": np.ascontiguousarray(np.asarray(b1).reshape(128, 1), dtype=np.float32),
        "b2": np.ascontiguousarray(np.asarray(b2).reshape(128, 1), dtype=np.float32),
        "b3x2": np.ascontiguousarray(
            (2.0 * np.asarray(b3)).reshape(2, 1), dtype=np.float32),
    }, slot


def kernel(h, src, dst, W1, b1, W2, b2, W3, b3, **run_kwargs):
    h = np.asarray(h, dtype=np.float32)
    src = np.asarray(src).astype(np.int64)
    dst = np.asarray(dst).astype(np.int64)
    W1 = np.asarray(W1); W2 = np.asarray(W2); W3 = np.asarray(W3)
    b1 = np.asarray(b1); b2 = np.asarray(b2); b3 = np.asarray(b3)

    h16 = np.zeros((N_PAD, D), dtype=np.float16)
    h16[:N_NODES] = h.astype(np.float16)

    nc = _get_nc()
    in_maps, slots = [], []
    for c in range(N_CORES):
        sl = slice(c * E_CORE, (c + 1) * E_CORE)
        im, slot = make_in_map(h16, src[sl], dst[sl], W1, b1, W2, b2, W3, b3)
        in_maps.append(im)
        slots.append(slot)

    try:
        res = run_bass_kernel_spmd(nc, in_maps, core_ids=list(range(N_CORES)),
                                   **run_kwargs)
    except Exception:
        import time as _time
        _time.sleep(5)
        res = run_bass_kernel_spmd(nc, in_maps, core_ids=list(range(N_CORES)),
                                   **run_kwargs)

    out = np.empty((E_TOTAL, 2), dtype=np.float32)
    for c in range(N_CORES):
        o = res.results[c]["out"]          # [2, E_PAD]
        out[c * E_CORE:(c + 1) * E_CORE] = o.T[slots[c]]
    if run_kwargs:
        kernel.last_results = res
    return out


# revision 17
# speedup vs baseline: 1.0444x; 1.0272x over previous
"""Split-side Trainium2 kernel for BondPoolingLayer.

Architecture (single SPMD program, per-core variation lives in data only):
  - Node space padded to 200704 rows = 56 blocks x 3584. Stream block b is
    one contiguous-per-partition DMA (partition p holds rows 3584b+28p..+28,
    7KB descriptors -> full DMA bandwidth). Chunk (b,j), j in [0,28), is the
    128-row set {3584b + 28p + j}; its chunk-local id of node n is the
    partition p. Band = 7 consecutive chunks (896 nodes), 4 bands/block,
    224 bands total.
  - Edges are binned by src band; each band owns 128 fixed edge slots
    (mean occupancy 123/128). Overflow edges go to 128-slot spill
    mini-tiles sized from the worst core's actual spill (~5 units for the
    seed-0 inputs). Slot layout is FIXED (program structure identical on
    all cores); which edge sits in which slot is data.
  - src side (banded slots): one-hot selection matmuls. S is built on-chip:
    DVE is_equal of the broadcast slot-request tile (req = 128*(j%7) + p,
    f16) against f32 scalars (128k + p). For tile t (= block t, 4 bands,
    512 slots): 28 matmuls, stationary H_chunk [128 nodes, 128 feat] fp16,
    rhs S slice [128, 128] -> feature-major psum. Zero Pool time.
  - dst side + spill src: SWDGE indirect DMA gather of h fp16 rows (240
    calls x 128 rows ~ 1.4us Pool each, vs 432 in the baseline), then PE
    transpose to feature-major.
  - MLP fp16 weights/acts, fp32 psum, per 512-slot tile. Host maps slots
    back to edges.
"""

import numpy as np

import concourse.bass as bass
import concourse.mybir as mybir
import concourse.tile as tile
from concourse import bacc
from concourse.bass import IndirectOffsetOnAxis
from concourse.bass_utils import run_bass_kernel_spmd
from concourse.masks import make_identity

N_NODES = 200000
D = 128
E_TOTAL = 220000
N_CORES = 8
E_CORE = E_TOTAL // N_CORES      # 27500

ROWS_PP = 28                      # rows per partition per block
BLK_ROWS = 128 * ROWS_PP          # 3584
N_BLOCKS = 56
N_PAD = N_BLOCKS * BLK_ROWS       # 200704
BAND_CH = 7                       # chunks per band
BANDS_PB = ROWS_PP // BAND_CH     # 4 bands per block
N_BANDS = N_BLOCKS * BANDS_PB     # 224
BAND_NODES = BAND_CH * 128        # 896

CAP = 128                         # edge slots per band
TILE_E = BANDS_PB * CAP           # 512 slots per tile
N_BTILES = N_BLOCKS               # 56 banded tiles
BANDED_SLOTS = N_BTILES * TILE_E  # 28672
SPILL_UNIT = 128                  # slots per spill mini-tile
DUMMY_REQ = 4096.0                # matches nothing (< 128*7 never)


def e_pad(n_units):
    return BANDED_SLOTS + n_units * SPILL_UNIT

F32 = mybir.dt.float32
F16 = mybir.dt.float16
I32 = mybir.dt.int32


def build_nc(n_units=5):
    nc = bacc.Bacc("TRN2", target_bir_lowering=False, debug=False)

    h = nc.dram_tensor("h", [N_PAD, D], F16, kind="ExternalInput")
    didx = nc.dram_tensor("didx", [128, e_pad(n_units) // 128], I32,
                          kind="ExternalInput")
    sidx = nc.dram_tensor("sidx", [128, n_units], I32,
                          kind="ExternalInput")
    sreq = nc.dram_tensor("sreq", [1, N_BTILES * TILE_E], F16,
                          kind="ExternalInput")
    kiota = nc.dram_tensor("kiota", [128, BAND_CH], F32, kind="ExternalInput")
    w1a = nc.dram_tensor("w1a", [128, 128], F16, kind="ExternalInput")
    w1b = nc.dram_tensor("w1b", [128, 128], F16, kind="ExternalInput")
    w2 = nc.dram_tensor("w2", [128, 128], F16, kind="ExternalInput")
    w3 = nc.dram_tensor("w3", [128, 2], F16, kind="ExternalInput")
    b1 = nc.dram_tensor("b1", [128, 1], F32, kind="ExternalInput")
    b2 = nc.dram_tensor("b2", [128, 1], F32, kind="ExternalInput")
    b3x2 = nc.dram_tensor("b3x2", [2, 1], F32, kind="ExternalInput")
    out = nc.dram_tensor("out", [2, e_pad(n_units)], F32, kind="ExternalOutput")

    with tile.TileContext(nc) as tc:
        _program(tc, nc, n_units, h, didx, sidx, sreq, kiota,
                 w1a, w1b, w2, w3, b1, b2, b3x2, out)
    nc.compile()
    return nc


def _program(tc, nc, n_units, h, didx, sidx, sreq, kiota,
             w1a, w1b, w2, w3, b1, b2, b3x2, out):
    Relu = mybir.ActivationFunctionType.Relu
    EQ = mybir.AluOpType.is_equal

    with (
        tc.tile_pool(name="const", bufs=1) as const_pool,
        tc.tile_pool(name="stream", bufs=3) as stream_pool,
        tc.tile_pool(name="req", bufs=2) as req_pool,
        tc.tile_pool(name="sel", bufs=2) as sel_pool,
        tc.tile_pool(name="gat", bufs=4) as gat_pool,
        tc.tile_pool(name="xbuf", bufs=6) as x_pool,
        tc.tile_pool(name="act", bufs=3) as act_pool,
        tc.tile_pool(name="outp", bufs=4) as out_pool,
        tc.tile_pool(name="xsp", bufs=2, space="PSUM") as xs_psum,
        tc.tile_pool(name="trp", bufs=2, space="PSUM") as tr_psum,
        tc.tile_pool(name="l1p", bufs=1, space="PSUM") as l1_psum,
        tc.tile_pool(name="l2p", bufs=1, space="PSUM") as l2_psum,
    ):
        # ---- constants ----
        ident = const_pool.tile([128, 128], F16)
        make_identity(nc, ident[:])
        w1a_t = const_pool.tile([128, 128], F16)
        nc.sync.dma_start(w1a_t[:], w1a.ap())
        w1b_t = const_pool.tile([128, 128], F16)
        nc.sync.dma_start(w1b_t[:], w1b.ap())
        w2_t = const_pool.tile([128, 128], F16)
        nc.sync.dma_start(w2_t[:], w2.ap())
        w3_t = const_pool.tile([128, 2], F16)
        nc.sync.dma_start(w3_t[:], w3.ap())
        b1_t = const_pool.tile([128, 1], F32)
        nc.sync.dma_start(b1_t[:], b1.ap())
        b2_t = const_pool.tile([128, 1], F32)
        nc.sync.dma_start(b2_t[:], b2.ap())
        b3_t = const_pool.tile([2, 1], F32)
        nc.sync.dma_start(b3_t[:], b3x2.ap())
        kio_t = const_pool.tile([128, BAND_CH], F32)
        nc.sync.dma_start(kio_t[:], kiota.ap())
        didx_t = const_pool.tile([128, e_pad(n_units) // 128], I32)
        nc.sync.dma_start(didx_t[:], didx.ap())
        sidx_t = const_pool.tile([128, n_units], I32)
        nc.sync.dma_start(sidx_t[:], sidx.ap())

        # DRAM h viewed per stream block: partition p <- rows 3584b+28p..+28
        h_blk = h.ap().rearrange("(b p k) d -> b p k d", p=128, k=ROWS_PP)
        sreq_ap = sreq.ap()
        out_ap = out.ap()

        def mlp_tile(col0, w, xsT, xdT):
            l1f = l1_psum.tile([128, TILE_E], F32, tag="l1f", space="PSUM")
            nc.tensor.matmul(l1f[:, :w], w1a_t[:], xsT, start=True, stop=False)
            nc.tensor.matmul(l1f[:, :w], w1b_t[:], xdT, start=False, stop=True)
            h1f = act_pool.tile([128, TILE_E], F16, tag="h1f")
            nc.scalar.activation(h1f[:, :w], l1f[:, :w], Relu, bias=b1_t[:, 0:1])

            l1r = l1_psum.tile([128, TILE_E], F32, tag="l1r", space="PSUM")
            nc.tensor.matmul(l1r[:, :w], w1a_t[:], xdT, start=True, stop=False)
            nc.tensor.matmul(l1r[:, :w], w1b_t[:], xsT, start=False, stop=True)
            h1r = act_pool.tile([128, TILE_E], F16, tag="h1r")
            nc.scalar.activation(h1r[:, :w], l1r[:, :w], Relu, bias=b1_t[:, 0:1])

            l2f = l2_psum.tile([128, TILE_E], F32, tag="l2f", space="PSUM")
            nc.tensor.matmul(l2f[:, :w], w2_t[:], h1f[:, :w], start=True, stop=True)
            h2f = act_pool.tile([128, TILE_E], F16, tag="h2f")
            nc.scalar.activation(h2f[:, :w], l2f[:, :w], Relu, bias=b2_t[:, 0:1])

            l2r = l2_psum.tile([128, TILE_E], F32, tag="l2r", space="PSUM")
            nc.tensor.matmul(l2r[:, :w], w2_t[:], h1r[:, :w], start=True, stop=True)
            h2r = act_pool.tile([128, TILE_E], F16, tag="h2r")
            nc.scalar.activation(h2r[:, :w], l2r[:, :w], Relu, bias=b2_t[:, 0:1])

            l3 = l2_psum.tile([2, TILE_E], F32, tag="l3", space="PSUM")
            nc.tensor.matmul(l3[:, :w], w3_t[:], h2f[:, :w], start=True, stop=False)
            nc.tensor.matmul(l3[:, :w], w3_t[:], h2r[:, :w], start=False, stop=True)

            o = out_pool.tile([2, TILE_E], F32, tag="o")
            nc.vector.tensor_scalar_add(o[:, :w], l3[:, :w], b3_t[:, 0:1])
            nc.sync.dma_start(out_ap[:, col0:col0 + w], o[:, :w])

        def gather_issue(idx_tile, col0, ncalls):
            gt = gat_pool.tile([128, 4, D], F16, tag="g")
            for gj in range(ncalls):
                nc.gpsimd.indirect_dma_start(
                    out=gt[:, gj, :],
                    out_offset=None,
                    in_=h.ap(),
                    in_offset=IndirectOffsetOnAxis(
                        ap=idx_tile[:, col0 + gj:col0 + gj + 1], axis=0),
                )
            return gt, ncalls

        def gather_consume(gt_n, dst_sb):
            gt, ncalls = gt_n
            trp = tr_psum.tile([128, TILE_E], F16, tag="tr", space="PSUM")
            for gj in range(ncalls):
                nc.tensor.transpose(
                    trp[:, gj * 128:(gj + 1) * 128], gt[:, gj, :], ident[:])
            nc.vector.tensor_copy(dst_sb[:, :ncalls * 128],
                                  trp[:, :ncalls * 128])

        # gather issue schedule: banded tiles first, spill last
        gplan = []
        for t in range(N_BTILES):
            gplan.append((didx_t, t * 4, 4))
        for u in range(n_units):
            gplan.append((sidx_t, u, 1))                       # spill src
            gplan.append((didx_t, BANDED_SLOTS // 128 + u, 1))  # spill dst
        PREFETCH = 2
        pending = {}
        for gi in range(PREFETCH):
            pending[gi] = gather_issue(*gplan[gi])
        next_gi = PREFETCH

        def fetch(gi, dst_sb):
            nonlocal next_gi
            gt = pending.pop(gi)
            if next_gi < len(gplan):
                pending[next_gi] = gather_issue(*gplan[next_gi])
                next_gi += 1
            gather_consume(gt, dst_sb)

        # ---- banded tiles ----
        for t in range(N_BTILES):
            st = stream_pool.tile([128, ROWS_PP, D], F16, tag="sb")
            nc.sync.dma_start(st[:], h_blk[t])

            rq = req_pool.tile([128, TILE_E], F16, tag="rq")
            nc.sync.dma_start(
                rq[:],
                sreq_ap[:, t * TILE_E:(t + 1) * TILE_E].broadcast_to(
                    [128, TILE_E]))

            s_t = sel_pool.tile([128, BAND_CH, TILE_E], F16, tag="s")
            for k in range(BAND_CH):
                nc.vector.tensor_scalar(s_t[:, k, :], rq[:], kio_t[:, k:k + 1],
                                        None, EQ)

            xd = x_pool.tile([128, TILE_E], F16, tag="xd")
            fetch(t, xd)

            xsp = xs_psum.tile([128, TILE_E], F32, tag="xsp", space="PSUM")
            for jg in range(BANDS_PB):
                for k in range(BAND_CH):
                    j = jg * BAND_CH + k
                    nc.tensor.matmul(
                        xsp[:, jg * CAP:(jg + 1) * CAP],
                        st[:, j, :],
                        s_t[:, k, jg * CAP:(jg + 1) * CAP],
                        start=(k == 0), stop=(k == BAND_CH - 1))
            xs = x_pool.tile([128, TILE_E], F16, tag="xs")
            nc.vector.tensor_copy(xs[:], xsp[:])

            mlp_tile(t * TILE_E, TILE_E, xs[:], xd[:])

        # ---- spill mini-tiles: both sides gathered ----
        for u in range(n_units):
            xs = x_pool.tile([128, TILE_E], F16, tag="xs")
            fetch(N_BTILES + 2 * u, xs)
            xd = x_pool.tile([128, TILE_E], F16, tag="xd")
            fetch(N_BTILES + 2 * u + 1, xd)
            mlp_tile(BANDED_SLOTS + u * SPILL_UNIT, SPILL_UNIT,
                     xs[:, :SPILL_UNIT], xd[:, :SPILL_UNIT])


_NC_CACHE = {}


def _get_nc(n_units):
    if n_units not in _NC_CACHE:
        _NC_CACHE[n_units] = build_nc(n_units)
    return _NC_CACHE[n_units]


def _node_meta(n):
    """(band, req_local) of node id n under the block/chunk layout."""
    b, r = divmod(n, BLK_ROWS)
    p, j = divmod(r, ROWS_PP)
    band = b * BANDS_PB + j // BAND_CH
    req = 128 * (j % BAND_CH) + p
    return band, req


def assign_slots(src):
    """Bin edges by src band into fixed 128-cap slots; overflow -> spill.
    Returns (slot_of_edge [n] with spill slots at BANDED_SLOTS.., req_local,
    n_spilled)."""
    n = len(src)
    band, req = _node_meta(src)
    slot = np.full(n, -1, dtype=np.int64)
    counts = np.zeros(N_BANDS, dtype=np.int64)
    spill = []
    for e in range(n):
        b = band[e]
        if counts[b] < CAP:
            slot[e] = b * CAP + counts[b]
            counts[b] += 1
        else:
            spill.append(e)
    for i, e in enumerate(spill):
        slot[e] = BANDED_SLOTS + i
    return slot, req, len(spill)


def _wrap_idx(vals, n_slots):
    """[n_slots] int -> [128, n_slots//128] int32 with tile j partition p
    holding vals[128*j + p]."""
    return np.ascontiguousarray(
        vals.reshape(n_slots // 128, 128).T.astype(np.int32))


def make_in_map(h16, src_shard, dst_shard, W1, b1, W2, b2, W3, b3, n_units=5):
    src = np.asarray(src_shard, dtype=np.int64)
    dst = np.asarray(dst_shard, dtype=np.int64)
    slot, req_local, n_spill = assign_slots(src)

    nb = BANDED_SLOTS
    spill_slots = n_units * SPILL_UNIT
    if n_spill > spill_slots:
        raise RuntimeError(f"spill overflow: {n_spill} > {spill_slots}")
    sreq = np.full(nb, DUMMY_REQ, dtype=np.float16)
    didx_full = np.zeros(e_pad(n_units), dtype=np.int64)
    sidx_spill = np.zeros(spill_slots, dtype=np.int64)

    banded = slot < nb
    sreq[slot[banded]] = req_local[banded].astype(np.float16)
    didx_full[slot] = dst
    sp = ~banded
    sidx_spill[slot[sp] - nb] = src[sp]

    kio = (np.arange(128, dtype=np.float32)[:, None]
           + 128.0 * np.arange(BAND_CH, dtype=np.float32)[None, :])

    return {
        "h": h16,
        "didx": _wrap_idx(didx_full, e_pad(n_units)),
        "sidx": _wrap_idx(sidx_spill, spill_slots),
        "sreq": np.ascontiguousarray(sreq.reshape(1, nb)),
        "kiota": np.ascontiguousarray(kio),
        "w1a": np.ascontiguousarray(W1[:128], dtype=np.float16),
        "w1b": np.ascontiguousarray(W1[128:], dtype=np.float16),
        "w2": np.ascontiguousarray(W2, dtype=np.float16),
        "w3": np.ascontiguousarray(W3, dtype=np.float16),
        "b1": np.ascontiguousarray(np.asarray(b1).reshape(128, 1), dtype=np.float32),
        "b2": np.ascontiguousarray(np.asarray(b2).reshape(128, 1), dtype=np.float32),
        "b3x2": np.ascontiguousarray(
            (2.0 * np.asarray(b3)).reshape(2, 1), dtype=np.float32),
    }, slot


def kernel(h, src, dst, W1, b1, W2, b2, W3, b3, **run_kwargs):
    h = np.asarray(h, dtype=np.float32)
    src = np.asarray(src).astype(np.int64)
    dst = np.asarray(dst).astype(np.int64)
    W1 = np.asarray(W1); W2 = np.asarray(W2); W3 = np.asarray(W3)
    b1 = np.asarray(b1); b2 = np.asarray(b2); b3 = np.asarray(b3)

    h16 = np.zeros((N_PAD, D), dtype=np.float16)
    h16[:N_NODES] = h.astype(np.float16)

    # size the spill region from the worst core (SPMD: one shared program)
    n_units = 1
    for c in range(N_CORES):
        sl = slice(c * E_CORE, (c + 1) * E_CORE)
        _, _, n_spill = assign_slots(src[sl])
        n_units = max(n_units, -(-n_spill // SPILL_UNIT))

    nc = _get_nc(n_units)
    in_maps, slots = [], []
    for c in range(N_CORES):
        sl = slice(c * E_CORE, (c + 1) * E_CORE)
        im, slot = make_in_map(h16, src[sl], dst[sl], W1, b1, W2, b2, W3, b3,
                               n_units=n_units)
        in_maps.append(im)
        slots.append(slot)

    try:
        res = run_bass_kernel_spmd(nc, in_maps, core_ids=list(range(N_CORES)),
                                   **run_kwargs)
    except Exception:
        import time as _time
        _time.sleep(5)
        res = run_bass_kernel_spmd(nc, in_maps, core_ids=list(range(N_CORES)),
                                   **run_kwargs)

    out = np.empty((E_TOTAL, 2), dtype=np.float32)
    for c in range(N_CORES):
        o = res.results[c]["out"]          # [2, E_PAD]
        out[c * E_CORE:(c + 1) * E_CORE] = o.T[slots[c]]
    if run_kwargs:
        kernel.last_results = res
    return out


# revision 33
# speedup vs baseline: 1.0728x; 1.0272x over previous
"""Split-side Trainium2 kernel for BondPoolingLayer.

Architecture (single SPMD program, per-core variation lives in data only):
  - Node space padded to 200704 rows = 56 blocks x 3584. Stream block b is
    one contiguous-per-partition DMA (partition p holds rows 3584b+28p..+28,
    7KB descriptors -> full DMA bandwidth). Chunk (b,j), j in [0,28), is the
    128-row set {3584b + 28p + j}; its chunk-local id of node n is the
    partition p. Band = 7 consecutive chunks (896 nodes), 4 bands/block,
    224 bands total.
  - Edges are binned by band with TWO choices: since the layer is symmetric
    (MLP(s||d) + MLP(d||s) is invariant under swapping src/dst), an edge can
    go to band(src) or band(dst) with roles swapped. Two-choice greedy
    balancing makes band overflow vanish (0 spilled edges on the reference
    inputs; 128-slot spill mini-tiles remain as a data-sized fallback).
    Slot layout is FIXED (program structure identical on all cores); which
    edge sits in which slot is data.
  - src side (banded slots): one-hot selection matmuls. S is built on-chip:
    DVE is_equal of the broadcast slot-request tile (req = 128*(j%7) + p,
    f16) against f32 scalars (128k + p). For tile t (= block t, 4 bands,
    512 slots): 28 matmuls, stationary H_chunk [128 nodes, 128 feat] fp16,
    rhs S slice [128, 128] -> feature-major psum. Zero Pool time.
  - dst side + spill src: SWDGE indirect DMA gather of h fp16 rows (240
    calls x 128 rows ~ 1.4us Pool each, vs 432 in the baseline), then PE
    transpose to feature-major.
  - MLP fp16 weights/acts, fp32 psum, per 512-slot tile. Host maps slots
    back to edges.
"""

import numpy as np

import concourse.bass as bass
import concourse.mybir as mybir
import concourse.tile as tile
from concourse import bacc
from concourse.bass import IndirectOffsetOnAxis
from concourse.bass_utils import run_bass_kernel_spmd

N_NODES = 200000
D = 128
E_TOTAL = 220000
N_CORES = 8
E_CORE = E_TOTAL // N_CORES      # 27500

ROWS_PP = 28                      # rows per partition per block
BLK_ROWS = 128 * ROWS_PP          # 3584
N_BLOCKS = 56
N_PAD = N_BLOCKS * BLK_ROWS       # 200704
BAND_CH = 7                       # chunks per band
BANDS_PB = ROWS_PP // BAND_CH     # 4 bands per block
N_BANDS = N_BLOCKS * BANDS_PB     # 224
BAND_NODES = BAND_CH * 128        # 896

CAP = 124                         # edge slots per band (two-choice keeps
                                  # overflow ~0 even at 99% mean occupancy)
TILE_E = BANDS_PB * CAP           # 496 slots per MLP tile
N_BTILES = N_BLOCKS               # 56 banded tiles
BANDED_SLOTS = N_BTILES * TILE_E  # 27776 (= 217 gather calls)
SPILL_UNIT = 128                  # slots per spill mini-tile
DUMMY_REQ = 4096.0                # matches nothing (< 128*7 never)


def e_pad(n_units):
    return BANDED_SLOTS + n_units * SPILL_UNIT

F32 = mybir.dt.float32
F16 = mybir.dt.float16
I32 = mybir.dt.int32


def build_nc(n_units=0):
    nc = bacc.Bacc("TRN2", target_bir_lowering=False, debug=False)

    h = nc.dram_tensor("h", [N_PAD, D], F16, kind="ExternalInput")
    didx = nc.dram_tensor("didx", [128, e_pad(n_units) // 128], I32,
                          kind="ExternalInput")
    sidx = (nc.dram_tensor("sidx", [128, n_units], I32, kind="ExternalInput")
            if n_units else None)
    sreq = nc.dram_tensor("sreq", [1, N_BTILES * TILE_E], F16,
                          kind="ExternalInput")
    identd = nc.dram_tensor("identd", [128, 128], F16, kind="ExternalInput")
    kiota = nc.dram_tensor("kiota", [128, BAND_CH], F32, kind="ExternalInput")
    w1a = nc.dram_tensor("w1a", [128, 128], F16, kind="ExternalInput")
    w1b = nc.dram_tensor("w1b", [128, 128], F16, kind="ExternalInput")
    w2 = nc.dram_tensor("w2", [128, 128], F16, kind="ExternalInput")
    w3 = nc.dram_tensor("w3", [128, 2], F16, kind="ExternalInput")
    b1 = nc.dram_tensor("b1", [128, 1], F32, kind="ExternalInput")
    b2 = nc.dram_tensor("b2", [128, 1], F32, kind="ExternalInput")
    b3x2 = nc.dram_tensor("b3x2", [2, 1], F32, kind="ExternalInput")
    out = nc.dram_tensor("out", [2, e_pad(n_units)], F32, kind="ExternalOutput")

    with tile.TileContext(nc) as tc:
        _program(tc, nc, n_units, h, didx, sidx, sreq, identd, kiota,
                 w1a, w1b, w2, w3, b1, b2, b3x2, out)
    nc.compile()
    return nc


def _program(tc, nc, n_units, h, didx, sidx, sreq, identd, kiota,
             w1a, w1b, w2, w3, b1, b2, b3x2, out):
    Relu = mybir.ActivationFunctionType.Relu
    EQ = mybir.AluOpType.is_equal

    with (
        tc.tile_pool(name="const", bufs=1) as const_pool,
        tc.tile_pool(name="stream", bufs=3) as stream_pool,
        tc.tile_pool(name="req", bufs=2) as req_pool,
        tc.tile_pool(name="sel", bufs=2) as sel_pool,
        tc.tile_pool(name="gat", bufs=4) as gat_pool,
        tc.tile_pool(name="xall", bufs=1) as xall_pool,
        tc.tile_pool(name="xbuf", bufs=6) as x_pool,
        tc.tile_pool(name="act", bufs=3) as act_pool,
        tc.tile_pool(name="outp", bufs=4) as out_pool,
        tc.tile_pool(name="xsp", bufs=2, space="PSUM") as xs_psum,
        tc.tile_pool(name="trp", bufs=2, space="PSUM") as tr_psum,
        tc.tile_pool(name="l1p", bufs=1, space="PSUM") as l1_psum,
        tc.tile_pool(name="l2p", bufs=1, space="PSUM") as l2_psum,
    ):
        # ---- constants (didx first: Pool's gathers wait only on it) ----
        didx_t = const_pool.tile([128, e_pad(n_units) // 128], I32)
        nc.sync.dma_start(didx_t[:], didx.ap())
        if n_units:
            sidx_t = const_pool.tile([128, n_units], I32)
            nc.sync.dma_start(sidx_t[:], sidx.ap())
        else:
            sidx_t = None
        ident = const_pool.tile([128, 128], F16)
        nc.sync.dma_start(ident[:], identd.ap())
        w1a_t = const_pool.tile([128, 128], F16)
        nc.sync.dma_start(w1a_t[:], w1a.ap())
        w1b_t = const_pool.tile([128, 128], F16)
        nc.sync.dma_start(w1b_t[:], w1b.ap())
        w2_t = const_pool.tile([128, 128], F16)
        nc.sync.dma_start(w2_t[:], w2.ap())
        w3_t = const_pool.tile([128, 2], F16)
        nc.sync.dma_start(w3_t[:], w3.ap())
        b1_t = const_pool.tile([128, 1], F32)
        nc.sync.dma_start(b1_t[:], b1.ap())
        b2_t = const_pool.tile([128, 1], F32)
        nc.sync.dma_start(b2_t[:], b2.ap())
        b3_t = const_pool.tile([2, 1], F32)
        nc.sync.dma_start(b3_t[:], b3x2.ap())
        kio_t = const_pool.tile([128, BAND_CH], F32)
        nc.sync.dma_start(kio_t[:], kiota.ap())

        # DRAM h viewed per stream block: partition p <- rows 3584b+28p..+28
        h_blk = h.ap().rearrange("(b p k) d -> b p k d", p=128, k=ROWS_PP)
        sreq_ap = sreq.ap()
        out_ap = out.ap()

        def mlp_tile(col0, w, xsT, xdT):
            l1f = l1_psum.tile([128, TILE_E], F32, tag="l1f", space="PSUM")
            nc.tensor.matmul(l1f[:, :w], w1a_t[:], xsT, start=True, stop=False)
            nc.tensor.matmul(l1f[:, :w], w1b_t[:], xdT, start=False, stop=True)
            h1f = act_pool.tile([128, TILE_E], F16, tag="h1f")
            nc.scalar.activation(h1f[:, :w], l1f[:, :w], Relu, bias=b1_t[:, 0:1])

            l1r = l1_psum.tile([128, TILE_E], F32, tag="l1r", space="PSUM")
            nc.tensor.matmul(l1r[:, :w], w1a_t[:], xdT, start=True, stop=False)
            nc.tensor.matmul(l1r[:, :w], w1b_t[:], xsT, start=False, stop=True)
            h1r = act_pool.tile([128, TILE_E], F16, tag="h1r")
            nc.scalar.activation(h1r[:, :w], l1r[:, :w], Relu, bias=b1_t[:, 0:1])

            l2f = l2_psum.tile([128, TILE_E], F32, tag="l2f", space="PSUM")
            nc.tensor.matmul(l2f[:, :w], w2_t[:], h1f[:, :w], start=True, stop=True)
            h2f = act_pool.tile([128, TILE_E], F16, tag="h2f")
            nc.scalar.activation(h2f[:, :w], l2f[:, :w], Relu, bias=b2_t[:, 0:1])

            l2r = l2_psum.tile([128, TILE_E], F32, tag="l2r", space="PSUM")
            nc.tensor.matmul(l2r[:, :w], w2_t[:], h1r[:, :w], start=True, stop=True)
            h2r = act_pool.tile([128, TILE_E], F16, tag="h2r")
            nc.scalar.activation(h2r[:, :w], l2r[:, :w], Relu, bias=b2_t[:, 0:1])

            l3 = l2_psum.tile([2, TILE_E], F32, tag="l3", space="PSUM")
            nc.tensor.matmul(l3[:, :w], w3_t[:], h2f[:, :w], start=True, stop=False)
            nc.tensor.matmul(l3[:, :w], w3_t[:], h2r[:, :w], start=False, stop=True)

            o = out_pool.tile([2, TILE_E], F32, tag="o")
            nc.vector.tensor_scalar_add(o[:, :w], l3[:, :w], b3_t[:, 0:1])
            nc.sync.dma_start(out_ap[:, col0:col0 + w], o[:, :w])

        def gather_issue(idx_tile, col0, ncalls):
            gt = gat_pool.tile([128, 4, D], F16, tag="g")
            for gj in range(ncalls):
                nc.gpsimd.indirect_dma_start(
                    out=gt[:, gj, :],
                    out_offset=None,
                    in_=h.ap(),
                    in_offset=IndirectOffsetOnAxis(
                        ap=idx_tile[:, col0 + gj:col0 + gj + 1], axis=0),
                )
            return gt, ncalls

        # persistent gathered-side buffer; gather groups (128-slot granular)
        # are decoupled from MLP tiles (496-slot granular) via region deps
        xd_all = xall_pool.tile([128, e_pad(n_units)], F16)

        def gather_consume(gt_n, dst_ap):
            gt, ncalls = gt_n
            trp = tr_psum.tile([128, 512], F16, tag="tr", space="PSUM")
            for gj in range(ncalls):
                nc.tensor.transpose(
                    trp[:, gj * 128:(gj + 1) * 128], gt[:, gj, :], ident[:])
            nc.vector.tensor_copy(dst_ap, trp[:, :ncalls * 128])

        # gather issue schedule: banded groups (<=4 calls). The LAST few
        # tiles' groups are gathered FIRST (xd_all is persistent, so issue
        # order is free): at Pool-end only mid tiles remain, collapsing the
        # compute drain.
        nb_calls = BANDED_SLOTS // 128               # 217
        gplan = []
        for g0 in range(0, nb_calls, 4):
            ncall = min(4, nb_calls - g0)
            gplan.append((didx_t, g0, ncall, g0 * 128))
        n_bgroups = len(gplan)                       # 55
        EARLY = 4
        gplan = gplan[-EARLY:] + gplan[:-EARLY]
        for u in range(n_units):
            gplan.append((sidx_t, u, 1, None))        # spill src -> xs tile
            gplan.append((didx_t, nb_calls + u, 1,
                          BANDED_SLOTS + u * SPILL_UNIT))
        PREFETCH = 2
        pending = {}
        for gi in range(PREFETCH):
            pending[gi] = gather_issue(*gplan[gi][:3])
        next_gi = PREFETCH

        def fetch(gi, xs_tile=None):
            nonlocal next_gi
            gt = pending.pop(gi)
            if next_gi < len(gplan):
                pending[next_gi] = gather_issue(*gplan[next_gi][:3])
                next_gi += 1
            col0 = gplan[gi][3]
            if col0 is None:
                gather_consume(gt, xs_tile[:, :gt[1] * 128])
            else:
                gather_consume(gt, xd_all[:, col0:col0 + gt[1] * 128])

        # ---- banded tiles ----
        for t in range(N_BTILES):
            st = stream_pool.tile([128, ROWS_PP, D], F16, tag="sb")
            nc.sync.dma_start(st[:], h_blk[t])

            rq = req_pool.tile([128, TILE_E], F16, tag="rq")
            nc.sync.dma_start(
                rq[:],
                sreq_ap[:, t * TILE_E:(t + 1) * TILE_E].broadcast_to(
                    [128, TILE_E]))

            s_t = sel_pool.tile([128, BAND_CH, TILE_E], F16, tag="s")
            for k in range(BAND_CH):
                nc.vector.tensor_scalar(s_t[:, k, :], rq[:], kio_t[:, k:k + 1],
                                        None, EQ)

            if t < n_bgroups:
                fetch(t)

            xsp = xs_psum.tile([128, TILE_E], F32, tag="xsp", space="PSUM")
            for jg in range(BANDS_PB):
                for k in range(BAND_CH):
                    j = jg * BAND_CH + k
                    nc.tensor.matmul(
                        xsp[:, jg * CAP:(jg + 1) * CAP],
                        st[:, j, :],
                        s_t[:, k, jg * CAP:(jg + 1) * CAP],
                        start=(k == 0), stop=(k == BAND_CH - 1))
            xs = x_pool.tile([128, TILE_E], F16, tag="xs")
            nc.vector.tensor_copy(xs[:], xsp[:])

            mlp_tile(t * TILE_E, TILE_E, xs[:],
                     xd_all[:, t * TILE_E:(t + 1) * TILE_E])

        # ---- spill mini-tiles: src gathered to xs, dst in xd_all ----
        for u in range(n_units):
            xs = x_pool.tile([128, TILE_E], F16, tag="xs")
            fetch(n_bgroups + 2 * u, xs)
            fetch(n_bgroups + 2 * u + 1)
            col0 = BANDED_SLOTS + u * SPILL_UNIT
            mlp_tile(col0, SPILL_UNIT, xs[:, :SPILL_UNIT],
                     xd_all[:, col0:col0 + SPILL_UNIT])


_NC_CACHE = {}


def _get_nc(n_units):
    if n_units not in _NC_CACHE:
        _NC_CACHE[n_units] = build_nc(n_units)
    return _NC_CACHE[n_units]


def _node_meta(n):
    """(band, req_local) of node id n under the block/chunk layout."""
    b, r = divmod(n, BLK_ROWS)
    p, j = divmod(r, ROWS_PP)
    band = b * BANDS_PB + j // BAND_CH
    req = 128 * (j % BAND_CH) + p
    return band, req


def assign_slots(src, dst):
    """Two-choice banded slot assignment. The layer is symmetric under
    swapping (src, dst), so an edge may occupy a slot in band(src) with
    roles (sel, gat) = (src, dst), or in band(dst) with roles swapped.
    Greedy lower-load choice + one-step augmentation for the stragglers;
    residual overflow -> spill (essentially never). Returns
    (slot [n], sel [n], gat [n], n_spilled)."""
    n = len(src)
    band_s, _ = _node_meta(src)
    band_d, _ = _node_meta(dst)
    # choice[e]: 0 = slot in band_s (sel=src), 1 = slot in band_d (swapped)
    choice = np.zeros(n, dtype=np.int8)
    counts = np.zeros(N_BANDS, dtype=np.int64)
    members = [[] for _ in range(N_BANDS)]
    spill = []
    for e in range(n):
        a, b = band_s[e], band_d[e]
        ch = 0
        if counts[a] > counts[b]:
            a, b = b, a
            ch = 1
        if counts[a] >= CAP:
            if counts[b] >= CAP:
                spill.append(e)
                continue
            a = b
            ch ^= 1
        counts[a] += 1
        members[a].append(e)
        choice[e] = ch
    # augmentation: free a slot by relocating an occupant to ITS other band
    still = []
    for e in spill:
        placed = False
        for b_try, ch in ((band_s[e], 0), (band_d[e], 1)):
            for e2 in members[b_try]:
                ob = band_d[e2] if choice[e2] == 0 else band_s[e2]
                if ob != b_try and counts[ob] < CAP:
                    members[b_try].remove(e2)
                    members[ob].append(e2)
                    counts[ob] += 1
                    choice[e2] ^= 1
                    members[b_try].append(e)
                    choice[e] = ch
                    placed = True
                    break
            if placed:
                break
        if not placed:
            still.append(e)
    # assign slot numbers
    slot = np.full(n, -1, dtype=np.int64)
    for b in range(N_BANDS):
        for i, e in enumerate(members[b]):
            slot[e] = b * CAP + i
    for i, e in enumerate(still):
        slot[e] = BANDED_SLOTS + i
    swapped = choice.astype(bool)
    sel = np.where(swapped, dst, src)
    gat = np.where(swapped, src, dst)
    return slot, sel, gat, len(still)


def _wrap_idx(vals, n_slots):
    """[n_slots] int -> [128, n_slots//128] int32 with tile j partition p
    holding vals[128*j + p]."""
    return np.ascontiguousarray(
        vals.reshape(n_slots // 128, 128).T.astype(np.int32))


def make_in_map(h16, src_shard, dst_shard, W1, b1, W2, b2, W3, b3, n_units=0):
    src = np.asarray(src_shard, dtype=np.int64)
    dst = np.asarray(dst_shard, dtype=np.int64)
    slot, sel, gat, n_spill = assign_slots(src, dst)
    _, req_local = _node_meta(sel)

    nb = BANDED_SLOTS
    spill_slots = n_units * SPILL_UNIT
    if n_spill > spill_slots:
        raise RuntimeError(f"spill overflow: {n_spill} > {spill_slots}")
    sreq = np.full(nb, DUMMY_REQ, dtype=np.float16)
    didx_full = np.zeros(e_pad(n_units), dtype=np.int64)
    sidx_spill = np.zeros(max(spill_slots, 1), dtype=np.int64)

    banded = slot < nb
    sreq[slot[banded]] = req_local[banded].astype(np.float16)
    didx_full[slot] = gat
    sp = ~banded
    sidx_spill[slot[sp] - nb] = sel[sp]

    kio = (np.arange(128, dtype=np.float32)[:, None]
           + 128.0 * np.arange(BAND_CH, dtype=np.float32)[None, :])

    in_map = {
        "h": h16,
        "didx": _wrap_idx(didx_full, e_pad(n_units)),
        "sreq": np.ascontiguousarray(sreq.reshape(1, nb)),
        "identd": np.eye(128, dtype=np.float16),
        "kiota": np.ascontiguousarray(kio),
        "w1a": np.ascontiguousarray(W1[:128], dtype=np.float16),
        "w1b": np.ascontiguousarray(W1[128:], dtype=np.float16),
        "w2": np.ascontiguousarray(W2, dtype=np.float16),
        "w3": np.ascontiguousarray(W3, dtype=np.float16),
        "b1": np.ascontiguousarray(np.asarray(b1).reshape(128, 1), dtype=np.float32),
        "b2": np.ascontiguousarray(np.asarray(b2).reshape(128, 1), dtype=np.float32),
        "b3x2": np.ascontiguousarray(
            (2.0 * np.asarray(b3)).reshape(2, 1), dtype=np.float32),
    }
    if n_units:
        in_map["sidx"] = _wrap_idx(sidx_spill, spill_slots)
    return in_map, slot


def kernel(h, src, dst, W1, b1, W2, b2, W3, b3, **run_kwargs):
    h = np.asarray(h, dtype=np.float32)
    src = np.asarray(src).astype(np.int64)
    dst = np.asarray(dst).astype(np.int64)
    W1 = np.asarray(W1); W2 = np.asarray(W2); W3 = np.asarray(W3)
    b1 = np.asarray(b1); b2 = np.asarray(b2); b3 = np.asarray(b3)

    h16 = np.zeros((N_PAD, D), dtype=np.float16)
    h16[:N_NODES] = h.astype(np.float16)

    # size the spill region from the worst core (SPMD: one shared program)
    n_units = 0
    for c in range(N_CORES):
        sl = slice(c * E_CORE, (c + 1) * E_CORE)
        _, _, _, n_spill = assign_slots(src[sl], dst[sl])
        n_units = max(n_units, -(-n_spill // SPILL_UNIT))

    nc = _get_nc(n_units)
    in_maps, slots = [], []
    for c in range(N_CORES):
        sl = slice(c * E_CORE, (c + 1) * E_CORE)
        im, slot = make_in_map(h16, src[sl], dst[sl], W1, b1, W2, b2, W3, b3,
                               n_units=n_units)
        in_maps.append(im)
        slots.append(slot)

    try:
        res = run_bass_kernel_spmd(nc, in_maps, core_ids=list(range(N_CORES)),
                                   **run_kwargs)
    except Exception:
        import time as _time
        _time.sleep(5)
        res = run_bass_kernel_spmd(nc, in_maps, core_ids=list(range(N_CORES)),
                                   **run_kwargs)

    out = np.empty((E_TOTAL, 2), dtype=np.float32)
    for c in range(N_CORES):
        o = res.results[c]["out"]          # [2, E_PAD]
        out[c * E_CORE:(c + 1) * E_CORE] = o.T[slots[c]]
    if run_kwargs:
        kernel.last_results = res
    return out
